# revision 8
# baseline (speedup 1.0000x reference)
"""GQA kernel for Trainium2, 8 NeuronCores.

Sharding: core c -> batch b = c//4, kv-head-group g = c%4.
Each core handles 1 batch, 2 KV heads (2g, 2g+1), 8 Q heads (8g..8g+7),
row-shard of W_o (rows 512g..512g+512). Host sums the 4 partial outputs
per batch and adds bo.

Device pipeline (all matmuls in fp32r: full-rate fp32 storage, ~1e-4 rel):
  1. QKV projections from x^T (E on partitions): out^T layout via
     stationary W chunks, moving x^T.  Q scaled by 1/8, biases folded.
  2. V^T transposed to V[t,d] via PE; a ones column is appended so the
     attention matmul also produces the softmax denominator (row 64).
  3. Flash-style attention per (kv, qh): S^T = K^T.T Q^T per k-block,
     causal via block skip + diag mask, exp on ACT (no max subtraction:
     |scores| <= ~7), A^T @ [V|1] accumulates out^T and l in PSUM.
  4. Normalize columns by 1/l via ones-outer-product broadcast.
  5. O-projection partial = attn^T.T @ Wo_shard, streamed to DRAM.
"""

import numpy as np

E = 2048
S = 2048
B = 2
D = 64
NCORE = 8
TGW = 256          # t-column group width in projections
NTG = S // TGW     # 8
EC = E // 128      # 16 contraction chunks
NKB = S // 128     # 16 key blocks
NQG = S // 512     # 4 psum column groups

_CACHE = {}
# tile jb holds q-heads (jb, jb+4): kv0 heads at base partition 0,
# kv1 heads at base partition 64, matching the K/V partition layout
HEAD_PERM = [0, 4, 1, 5, 2, 6, 3, 7]


def _build():
    import concourse.bass as bass
    import concourse.tile as tile
    from concourse import mybir, bacc
    from concourse.masks import make_identity

    F32 = mybir.dt.float32
    F32R = mybir.dt.float32r
    Exp = mybir.ActivationFunctionType.Exp
    Identity = mybir.ActivationFunctionType.Identity
    Copy = mybir.ActivationFunctionType.Copy

    nc = bacc.Bacc("TRN2", target_bir_lowering=False, debug=False,
                   num_devices=NCORE)

    XT = nc.declare_dram_parameter("xt", [E, S], F32, isOutput=False)
    WQ = nc.declare_dram_parameter("wq", [128, EC, 512], F32, isOutput=False)
    WK = nc.declare_dram_parameter("wk", [128, EC, 128], F32, isOutput=False)
    WV = nc.declare_dram_parameter("wv", [128, EC, 128], F32, isOutput=False)
    WO = nc.declare_dram_parameter("wo", [128, 4, E], F32, isOutput=False)
    BIAS = nc.declare_dram_parameter("bias", [128, 6], F32, isOutput=False)
    CM = nc.declare_dram_parameter("cmask", [128, 128], F32, isOutput=False)
    OUT = nc.declare_dram_parameter("out", [S, E], F32, isOutput=True)

    with tile.TileContext(nc) as tc:
        with tc.tile_pool(name="persist", bufs=1) as persist, \
             tc.tile_pool(name="xr", bufs=2) as xrp:

            qt = [persist.tile([128, S], F32R, tag=f"qt{i}", name=f"qt{i}") for i in range(4)]
            kt = persist.tile([128, S], F32R, tag="kt")
            v_t = persist.tile([128, 2, NKB, D + 1], F32R, tag="v")
            attn = [persist.tile([128, S], F32R, tag=f"attn{i}", name=f"attn{i}") for i in range(4)]
            cm = persist.tile([128, 128], F32, tag="cm")
            bias_t = persist.tile([128, 6], F32, tag="bias")
            ident = persist.tile([128, 64], F32, tag="ident")
            ones_row = persist.tile([1, 64], F32, tag="ones_row")
            ones64 = persist.tile([1, 64], F32R, tag="ones64")
            ones_col = persist.tile([128, 1], F32, tag="ones_col")

            nc.sync.dma_start(out=cm, in_=CM[:, :])
            nc.sync.dma_start(out=bias_t, in_=BIAS[:, :])
            make_identity(nc, ident[0:64, :])
            make_identity(nc, ident[64:128, :])
            nc.vector.memset(ones_row, 1.0)
            nc.scalar.activation(ones64, ones_row, Copy)
            nc.vector.memset(ones_col, 1.0)

            # ---------------- Phase 1: projections ----------------
            with tc.tile_pool(name="wts", bufs=1) as wts, \
                 tc.tile_pool(name="vtp", bufs=2) as vtp, \
                 tc.tile_pool(name="pp", bufs=3, space="PSUM") as pp, \
                 tc.tile_pool(name="tp", bufs=2, space="PSUM") as tpp:
                wq_r = wts.tile([128, EC, 512], F32R, tag="wq")
                for half in range(2):
                    wstg = xrp.tile([128, 8, 512], F32, tag="xr", name="wstg")
                    nc.sync.dma_start(out=wstg,
                                      in_=WQ[:, half * 8:(half + 1) * 8, :])
                    nc.vector.tensor_copy(wq_r[:, half * 8:(half + 1) * 8, :],
                                          wstg)
                wk_r = wts.tile([128, EC, 128], F32R, tag="wk")
                kstg = xrp.tile([128, EC, 128], F32, tag="xr", name="kstg")
                nc.sync.dma_start(out=kstg, in_=WK[:, :, :])
                nc.vector.tensor_copy(wk_r, kstg)
                wv_r = wts.tile([128, EC, 128], F32R, tag="wv")
                vstg = xrp.tile([128, EC, 128], F32, tag="xr", name="vstg")
                nc.sync.dma_start(out=vstg, in_=WV[:, :, :])
                nc.vector.tensor_copy(wv_r, vstg)

                xt_view = XT[:, :].rearrange("(ec p) t -> p ec t", p=128)

                for tg in range(NTG):
                    cols = bass.ds(tg * TGW, TGW)
                    xstg = xrp.tile([128, EC, TGW], F32, tag="xr", name="xstg")
                    nc.sync.dma_start(out=xstg, in_=xt_view[:, :, cols])
                    xr = xrp.tile([128, EC, TGW], F32R, tag="xr")
                    nc.vector.tensor_copy(xr, xstg)
                    for jb in range(6):
                        ps = pp.tile([128, TGW], F32, tag="proj")
                        for ec in range(EC):
                            if jb < 4:
                                lhsT = wq_r[:, ec, jb * 128:(jb + 1) * 128]
                            elif jb == 4:
                                lhsT = wk_r[:, ec, :]
                            else:
                                lhsT = wv_r[:, ec, :]
                            nc.tensor.matmul(ps, lhsT, xr[:, ec, :],
                                             start=(ec == 0), stop=(ec == EC - 1))
                        if jb < 4:
                            nc.scalar.activation(qt[jb][:, cols], ps, Identity,
                                                 bias=bias_t[:, jb:jb + 1],
                                                 scale=0.125)
                        elif jb == 4:
                            nc.scalar.activation(kt[:, cols], ps, Identity,
                                                 bias=bias_t[:, 4:5])
                        else:
                            vt = vtp.tile([128, TGW], F32, tag="vt")
                            nc.scalar.activation(vt, ps, Identity,
                                                 bias=bias_t[:, 5:6])
                            for kv in range(2):
                                for tc2 in range(TGW // 128):
                                    tps = tpp.tile([128, 64], F32, tag="tr")
                                    nc.tensor.transpose(
                                        tps,
                                        vt[kv * 64:kv * 64 + 64,
                                           tc2 * 128:(tc2 + 1) * 128],
                                        ident[kv * 64:kv * 64 + 64, :])
                                    kc = tg * (TGW // 128) + tc2
                                    nc.scalar.activation(
                                        v_t[:, kv, kc, 0:D], tps, Copy)
                                    nc.scalar.activation(
                                        v_t[:, kv, kc, D:D + 1], ones_col, Copy)

            # ---------------- Phase 2: attention ----------------
            with tc.tile_pool(name="at", bufs=4) as atp, \
                 tc.tile_pool(name="lp", bufs=2) as lp, \
                 tc.tile_pool(name="rsb", bufs=2) as rsb, \
                 tc.tile_pool(name="st", bufs=2, space="PSUM") as stp, \
                 tc.tile_pool(name="av", bufs=4, space="PSUM") as avp, \
                 tc.tile_pool(name="rp", bufs=1, space="PSUM") as rpp:
                for kv in range(2):
                    for hh in range(4):
                        qtile = qt[hh]
                        qoff = kv * 64
                        av = [avp.tile([D + 1, 512], F32, tag="av", name="av")
                              for _ in range(NQG)]
                        for kb in range(NKB):
                            qs = kb * 128
                            c = qs
                            while c < S:
                                w = min(512 - (c % 512), S - c)
                                st = stp.tile([128, 512], F32, tag="st")
                                nc.tensor.matmul(
                                    st[:, :w],
                                    kt[kv * 64:kv * 64 + 64, qs:qs + 128],
                                    qtile[qoff:qoff + 64, c:c + w],
                                    start=True, stop=True,
                                    skip_group_check=True)
                                if c == qs:
                                    nc.vector.tensor_add(st[:, 0:128],
                                                         st[:, 0:128], cm)
                                at = atp.tile([128, 512], F32R, tag="at")
                                nc.scalar.activation(at[:, :w], st[:, :w], Exp)
                                qg = c // 512
                                off = c % 512
                                nc.tensor.matmul(
                                    av[qg][:, off:off + w],
                                    v_t[:, kv, kb, :],
                                    at[:, :w],
                                    start=(kb == 0), stop=(kb == qg * 4 + 3),
                                    skip_group_check=True)
                                c += w
                        for qg in range(NQG):
                            lrow = lp.tile([1, 512], F32R, tag="lrow")
                            nc.scalar.activation(lrow, av[qg][D:D + 1, :], Copy)
                            rp = rpp.tile([64, 512], F32, tag="rp")
                            nc.tensor.matmul(rp, ones64, lrow,
                                             start=True, stop=True,
                                             skip_group_check=True)
                            rp_s = rsb.tile([64, 512], F32, tag="rp_s")
                            nc.vector.reciprocal(rp_s, rp)
                            dst = attn[hh][qoff:qoff + 64,
                                           qg * 512:(qg + 1) * 512]
                            nc.vector.tensor_mul(dst, av[qg][0:D, :], rp_s)

            # ---------------- Phase 3: output projection ----------------
            with tc.tile_pool(name="wo", bufs=1) as wop, \
                 tc.tile_pool(name="ostage", bufs=2) as osp, \
                 tc.tile_pool(name="op", bufs=4, space="PSUM") as opp:
                wo_r = wop.tile([128, 4, E], F32R, tag="wo")
                for half in range(2):
                    ostg = xrp.tile([128, 2, E], F32, tag="xr", name="ostg")
                    nc.sync.dma_start(out=ostg,
                                      in_=WO[:, half * 2:(half + 1) * 2, :])
                    nc.vector.tensor_copy(wo_r[:, half * 2:(half + 1) * 2, :],
                                          ostg)
                for tb in range(S // 128):
                    ops = [opp.tile([128, 512], F32, tag="op", name="op")
                           for _ in range(4)]
                    for jc in range(4):
                        lhsT = attn[jc][:, tb * 128:(tb + 1) * 128]
                        for ng in range(4):
                            nc.tensor.matmul(ops[ng], lhsT,
                                             wo_r[:, jc, ng * 512:(ng + 1) * 512],
                                             start=(jc == 0), stop=(jc == 3),
                                             skip_group_check=True)
                    ostage = osp.tile([128, E], F32, tag="ostage")
                    for ng in range(4):
                        nc.scalar.copy(ostage[:, ng * 512:(ng + 1) * 512],
                                       ops[ng])
                    nc.sync.dma_start(out=OUT[tb * 128:(tb + 1) * 128, :],
                                      in_=ostage)

    nc.compile()
    return nc


def _prep_core_inputs(c, x, Wq, bq, Wk, bk, Wv, bv, Wo, xt_cache):
    g = c % 4
    b = c // 4
    if b not in xt_cache:
        xt_cache[b] = np.ascontiguousarray(x[b].T).astype(np.float32)
    wq_s = Wq[:, 512 * g:512 * (g + 1)].reshape(E, 8, 64)
    wq_s = wq_s[:, HEAD_PERM, :].reshape(E, 512)
    wq = np.ascontiguousarray(
        wq_s.reshape(EC, 128, 512).transpose(1, 0, 2))
    wk = np.ascontiguousarray(
        Wk[:, 128 * g:128 * (g + 1)].reshape(EC, 128, 128).transpose(1, 0, 2))
    wv = np.ascontiguousarray(
        Wv[:, 128 * g:128 * (g + 1)].reshape(EC, 128, 128).transpose(1, 0, 2))
    wo_s = Wo[512 * g:512 * (g + 1), :].reshape(8, 64, E)
    wo_s = wo_s[HEAD_PERM, :, :].reshape(512, E)
    wo = np.ascontiguousarray(
        wo_s.reshape(4, 128, E).transpose(1, 0, 2))
    bias = np.zeros((128, 6), np.float32)
    bq_s = bq[512 * g:512 * (g + 1)].reshape(8, 64)[HEAD_PERM, :].reshape(512)
    bias[:, 0:4] = bq_s.reshape(4, 128).T * 0.125
    bias[:, 4] = bk[128 * g:128 * (g + 1)]
    bias[:, 5] = bv[128 * g:128 * (g + 1)]
    cmask = np.where(np.triu(np.ones((128, 128), bool)), 0.0,
                     -30000.0).astype(np.float32)
    return {"xt": xt_cache[b], "wq": wq, "wk": wk, "wv": wv, "wo": wo,
            "bias": bias, "cmask": cmask}


def kernel(**inputs):
    from concourse.bass_utils import run_bass_kernel_spmd

    x = np.asarray(inputs["x"], np.float32)
    Wq = np.asarray(inputs["Wq"], np.float32)
    bq = np.asarray(inputs["bq"], np.float32)
    Wk = np.asarray(inputs["Wk"], np.float32)
    bk = np.asarray(inputs["bk"], np.float32)
    Wv = np.asarray(inputs["Wv"], np.float32)
    bv = np.asarray(inputs["bv"], np.float32)
    Wo = np.asarray(inputs["Wo"], np.float32)
    bo = np.asarray(inputs["bo"], np.float32)

    if "nc" not in _CACHE:
        _CACHE["nc"] = _build()
    nc = _CACHE["nc"]

    xt_cache = {}
    in_maps = [_prep_core_inputs(c, x, Wq, bq, Wk, bk, Wv, bv, Wo, xt_cache)
               for c in range(NCORE)]
    res = run_bass_kernel_spmd(nc, in_maps, list(range(NCORE)))
    parts = [res.results[c]["out"] for c in range(NCORE)]
    out0 = parts[0] + parts[1] + parts[2] + parts[3] + bo
    out1 = parts[4] + parts[5] + parts[6] + parts[7] + bo
    return np.stack([out0, out1]).astype(np.float32)


# revision 11
# speedup vs baseline: 1.1086x; 1.1086x over previous
"""GQA kernel for Trainium2, 8 NeuronCores.

Sharding: core c -> batch b = c//4, kv-head-group g = c%4.
Each core handles 1 batch, 2 KV heads (2g, 2g+1), 8 Q heads (8g..8g+7),
row-shard of W_o (rows 512g..512g+512). Host sums the 4 partial outputs
per batch and adds bo.

Device pipeline (all matmuls in fp32r: full-rate fp32 storage, ~1e-4 rel):
  1. QKV projections from x^T (E on partitions): out^T layout via
     stationary W chunks, moving x^T.  Q scaled by 1/8, biases folded.
  2. V^T transposed to V[t,d] via PE; a ones column is appended so the
     attention matmul also produces the softmax denominator (row 64).
  3. Flash-style attention per (kv, qh): S^T = K^T.T Q^T per k-block,
     causal via block skip + diag mask, exp on ACT (no max subtraction:
     |scores| <= ~7), A^T @ [V|1] accumulates out^T and l in PSUM.
  4. Normalize columns by 1/l via ones-outer-product broadcast.
  5. O-projection partial = attn^T.T @ Wo_shard, streamed to DRAM.
"""

import numpy as np

E = 2048
S = 2048
B = 2
D = 64
NCORE = 8
TGW = 256          # t-column group width in projections
NTG = S // TGW     # 8
EC = E // 128      # 16 contraction chunks
NKB = S // 128     # 16 key blocks
NQG = S // 512     # 4 psum column groups

_CACHE = {}
# tile jb holds q-heads (jb, jb+4): kv0 heads at base partition 0,
# kv1 heads at base partition 64, matching the K/V partition layout
HEAD_PERM = [0, 4, 1, 5, 2, 6, 3, 7]


def _build():
    import concourse.bass as bass
    import concourse.tile as tile
    from concourse import mybir, bacc
    from concourse.masks import make_identity

    F32 = mybir.dt.float32
    F32R = mybir.dt.float32r
    Exp = mybir.ActivationFunctionType.Exp
    Identity = mybir.ActivationFunctionType.Identity
    Copy = mybir.ActivationFunctionType.Copy

    nc = bacc.Bacc("TRN2", target_bir_lowering=False, debug=False,
                   num_devices=NCORE)

    XT = nc.declare_dram_parameter("xt", [E, S], F32, isOutput=False)
    WQ = nc.declare_dram_parameter("wq", [128, EC, 512], F32, isOutput=False)
    WK = nc.declare_dram_parameter("wk", [128, EC, 128], F32, isOutput=False)
    WV = nc.declare_dram_parameter("wv", [128, EC, 128], F32, isOutput=False)
    WO = nc.declare_dram_parameter("wo", [128, 4, E], F32, isOutput=False)
    BIAS = nc.declare_dram_parameter("bias", [128, 6], F32, isOutput=False)
    CM = nc.declare_dram_parameter("cmask", [128, 128], F32, isOutput=False)
    OUT = nc.declare_dram_parameter("out", [S, E], F32, isOutput=True)

    with tile.TileContext(nc) as tc:
        with tc.tile_pool(name="persist", bufs=1) as persist, \
             tc.tile_pool(name="xr", bufs=3) as xrp:

            qt = [persist.tile([128, S], F32R, tag=f"qt{i}", name=f"qt{i}") for i in range(4)]
            kt = persist.tile([128, S], F32R, tag="kt")
            v_t = persist.tile([128, 2, NKB, D + 1], F32R, tag="v")
            attn = [persist.tile([128, S], F32R, tag=f"attn{i}", name=f"attn{i}") for i in range(4)]
            cm = persist.tile([128, 128], F32, tag="cm")
            bias_t = persist.tile([128, 6], F32, tag="bias")
            ident = persist.tile([128, 64], F32, tag="ident")
            ones_row = persist.tile([1, 64], F32, tag="ones_row")
            ones64 = persist.tile([1, 64], F32R, tag="ones64")
            ones_col = persist.tile([128, 1], F32, tag="ones_col")

            nc.sync.dma_start(out=cm, in_=CM[:, :])
            nc.sync.dma_start(out=bias_t, in_=BIAS[:, :])
            make_identity(nc, ident[0:64, :])
            make_identity(nc, ident[64:128, :])
            nc.vector.memset(ones_row, 1.0)
            nc.scalar.activation(ones64, ones_row, Copy)
            nc.vector.memset(ones_col, 1.0)

            # ---------------- Phase 1: projections ----------------
            with tc.tile_pool(name="wts", bufs=1) as wts, \
                 tc.tile_pool(name="vtp", bufs=2) as vtp, \
                 tc.tile_pool(name="pp", bufs=3, space="PSUM") as pp:
                wq_r = wts.tile([128, EC, 512], F32R, tag="wq")
                for half in range(2):
                    wstg = xrp.tile([128, 8, 512], F32, tag="xr", name="wstg")
                    nc.sync.dma_start(out=wstg,
                                      in_=WQ[:, half * 8:(half + 1) * 8, :])
                    nc.vector.tensor_copy(wq_r[:, half * 8:(half + 1) * 8, :],
                                          wstg)
                wk_r = wts.tile([128, EC, 128], F32R, tag="wk")
                kstg = xrp.tile([128, EC, 128], F32, tag="xr", name="kstg")
                nc.sync.dma_start(out=kstg, in_=WK[:, :, :])
                nc.vector.tensor_copy(wk_r, kstg)
                wv_r = wts.tile([128, EC, 128], F32R, tag="wv")
                vstg = xrp.tile([128, EC, 128], F32, tag="xr", name="vstg")
                nc.sync.dma_start(out=vstg, in_=WV[:, :, :])
                nc.vector.tensor_copy(wv_r, vstg)

                xt_view = XT[:, :].rearrange("(ec p) t -> p ec t", p=128)

                for tg in range(NTG):
                    cols = bass.ds(tg * TGW, TGW)
                    xstg = xrp.tile([128, EC, TGW], F32, tag="xr", name="xstg")
                    nc.sync.dma_start(out=xstg, in_=xt_view[:, :, cols])
                    xr = xrp.tile([128, EC, TGW], F32R, tag="xr")
                    nc.vector.tensor_copy(xr, xstg)
                    for jb in range(6):
                        ps = pp.tile([128, TGW], F32, tag="proj")
                        for ec in range(EC):
                            if jb < 4:
                                lhsT = wq_r[:, ec, jb * 128:(jb + 1) * 128]
                            elif jb == 4:
                                lhsT = wk_r[:, ec, :]
                            else:
                                lhsT = wv_r[:, ec, :]
                            nc.tensor.matmul(ps, lhsT, xr[:, ec, :],
                                             start=(ec == 0), stop=(ec == EC - 1))
                        if jb < 4:
                            nc.vector.tensor_scalar_add(qt[jb][:, cols], ps,
                                                        bias_t[:, jb:jb + 1])
                        elif jb == 4:
                            nc.vector.tensor_scalar_add(kt[:, cols], ps,
                                                        bias_t[:, 4:5])
                        else:
                            vt = vtp.tile([128, TGW], F32, tag="vt")
                            nc.scalar.activation(vt, ps, Identity,
                                                 bias=bias_t[:, 5:6])
                            for kv in range(2):
                                for tc2 in range(TGW // 128):
                                    tps = pp.tile([128, 64], F32, tag="proj", name="tps")
                                    nc.tensor.transpose(
                                        tps,
                                        vt[kv * 64:kv * 64 + 64,
                                           tc2 * 128:(tc2 + 1) * 128],
                                        ident[kv * 64:kv * 64 + 64, :])
                                    kc = tg * (TGW // 128) + tc2
                                    nc.scalar.activation(
                                        v_t[:, kv, kc, 0:D], tps, Copy)
                                    nc.scalar.activation(
                                        v_t[:, kv, kc, D:D + 1], ones_col, Copy)

            # ------- Phase 2+3: attention + O-projection, qg-major -------
            with tc.tile_pool(name="at", bufs=4) as atp, \
                 tc.tile_pool(name="lp", bufs=2) as lp, \
                 tc.tile_pool(name="rsb", bufs=2) as rsb, \
                 tc.tile_pool(name="wo", bufs=1) as wop, \
                 tc.tile_pool(name="ostage", bufs=2) as osp, \
                 tc.tile_pool(name="st", bufs=3, space="PSUM") as stp, \
                 tc.tile_pool(name="av", bufs=2, space="PSUM") as avp, \
                 tc.tile_pool(name="op", bufs=2, space="PSUM") as opp:
                wo_r = wop.tile([128, 4, E], F32R, tag="wo")
                for half in range(2):
                    ostg = xrp.tile([128, 2, E], F32, tag="xr", name="ostg")
                    nc.sync.dma_start(out=ostg,
                                      in_=WO[:, half * 2:(half + 1) * 2, :])
                    nc.vector.tensor_copy(wo_r[:, half * 2:(half + 1) * 2, :],
                                          ostg)
                for qg in range(NQG):
                    q0 = qg * 512
                    nkb = qg * 4 + 4
                    for hh in range(4):
                        for kv in range(2):
                            qoff = kv * 64
                            av = avp.tile([D + 1, 512], F32, tag="av",
                                          name="av")
                            for kb in range(nkb):
                                c0 = max(kb * 128, q0)
                                w = q0 + 512 - c0
                                st = stp.tile([128, 512], F32, tag="st",
                                              name="st")
                                nc.tensor.matmul(
                                    st[:, :w],
                                    kt[qoff:qoff + 64,
                                       kb * 128:(kb + 1) * 128],
                                    qt[hh][qoff:qoff + 64, c0:c0 + w],
                                    start=True, stop=True,
                                    skip_group_check=True)
                                if kb * 128 >= q0:
                                    nc.vector.tensor_add(st[:, 0:128],
                                                         st[:, 0:128], cm)
                                at = atp.tile([128, 512], F32R, tag="at",
                                              name="at")
                                nc.scalar.activation(at[:, :w], st[:, :w], Exp)
                                nc.tensor.matmul(
                                    av[:, c0 - q0:c0 - q0 + w],
                                    v_t[:, kv, kb, :],
                                    at[:, :w],
                                    start=(kb == 0), stop=(kb == nkb - 1),
                                    skip_group_check=True)
                            lrow = lp.tile([1, 512], F32R, tag="lrow",
                                           name="lrow")
                            nc.scalar.activation(lrow, av[D:D + 1, :], Copy)
                            rp = opp.tile([64, 512], F32, tag="op", name="rp")
                            nc.tensor.matmul(rp, ones64, lrow,
                                             start=True, stop=True,
                                             skip_group_check=True)
                            rp_s = rsb.tile([64, 512], F32, tag="rp_s",
                                            name="rp_s")
                            nc.vector.reciprocal(rp_s, rp)
                            dst = attn[hh][qoff:qoff + 64, q0:q0 + 512]
                            nc.vector.tensor_mul(dst, av[0:D, :], rp_s)
                    # O-projection for this qg's four t-blocks
                    for tb in range(qg * 4, qg * 4 + 4):
                        ostage = osp.tile([128, E], F32, tag="ostage",
                                          name="ostage")
                        for half in range(2):
                            ops = [opp.tile([128, 512], F32, tag="op",
                                            name="op") for _ in range(2)]
                            for jc in range(4):
                                lhsT = attn[jc][:, tb * 128:(tb + 1) * 128]
                                for ngi in range(2):
                                    ng = half * 2 + ngi
                                    nc.tensor.matmul(
                                        ops[ngi], lhsT,
                                        wo_r[:, jc, ng * 512:(ng + 1) * 512],
                                        start=(jc == 0), stop=(jc == 3),
                                        skip_group_check=True)
                            for ngi in range(2):
                                ng = half * 2 + ngi
                                nc.vector.tensor_copy(
                                    ostage[:, ng * 512:(ng + 1) * 512],
                                    ops[ngi])
                        nc.sync.dma_start(
                            out=OUT[tb * 128:(tb + 1) * 128, :], in_=ostage)

    nc.compile()
    return nc


def _prep_core_inputs(c, x, Wq, bq, Wk, bk, Wv, bv, Wo, xt_cache):
    g = c % 4
    b = c // 4
    if b not in xt_cache:
        xt_cache[b] = np.ascontiguousarray(x[b].T).astype(np.float32)
    wq_s = Wq[:, 512 * g:512 * (g + 1)].reshape(E, 8, 64)
    wq_s = wq_s[:, HEAD_PERM, :].reshape(E, 512)
    wq = np.ascontiguousarray(
        wq_s.reshape(EC, 128, 512).transpose(1, 0, 2)) * np.float32(0.125)
    wk = np.ascontiguousarray(
        Wk[:, 128 * g:128 * (g + 1)].reshape(EC, 128, 128).transpose(1, 0, 2))
    wv = np.ascontiguousarray(
        Wv[:, 128 * g:128 * (g + 1)].reshape(EC, 128, 128).transpose(1, 0, 2))
    wo_s = Wo[512 * g:512 * (g + 1), :].reshape(8, 64, E)
    wo_s = wo_s[HEAD_PERM, :, :].reshape(512, E)
    wo = np.ascontiguousarray(
        wo_s.reshape(4, 128, E).transpose(1, 0, 2))
    bias = np.zeros((128, 6), np.float32)
    bq_s = bq[512 * g:512 * (g + 1)].reshape(8, 64)[HEAD_PERM, :].reshape(512)
    bias[:, 0:4] = bq_s.reshape(4, 128).T * 0.125
    bias[:, 4] = bk[128 * g:128 * (g + 1)]
    bias[:, 5] = bv[128 * g:128 * (g + 1)]
    cmask = np.where(np.triu(np.ones((128, 128), bool)), 0.0,
                     -30000.0).astype(np.float32)
    return {"xt": xt_cache[b], "wq": wq, "wk": wk, "wv": wv, "wo": wo,
            "bias": bias, "cmask": cmask}


def kernel(**inputs):
    from concourse.bass_utils import run_bass_kernel_spmd

    x = np.asarray(inputs["x"], np.float32)
    Wq = np.asarray(inputs["Wq"], np.float32)
    bq = np.asarray(inputs["bq"], np.float32)
    Wk = np.asarray(inputs["Wk"], np.float32)
    bk = np.asarray(inputs["bk"], np.float32)
    Wv = np.asarray(inputs["Wv"], np.float32)
    bv = np.asarray(inputs["bv"], np.float32)
    Wo = np.asarray(inputs["Wo"], np.float32)
    bo = np.asarray(inputs["bo"], np.float32)

    if "nc" not in _CACHE:
        _CACHE["nc"] = _build()
    nc = _CACHE["nc"]

    xt_cache = {}
    in_maps = [_prep_core_inputs(c, x, Wq, bq, Wk, bk, Wv, bv, Wo, xt_cache)
               for c in range(NCORE)]
    res = run_bass_kernel_spmd(nc, in_maps, list(range(NCORE)))
    parts = [res.results[c]["out"] for c in range(NCORE)]
    out0 = parts[0] + parts[1] + parts[2] + parts[3] + bo
    out1 = parts[4] + parts[5] + parts[6] + parts[7] + bo
    return np.stack([out0, out1]).astype(np.float32)
